# revision 46
# baseline (speedup 1.0000x reference)
"""CTRNN forward kernel v3 for Trainium2 (8 NeuronCores).

Model (per step t):
    pre = x_t @ w_in^T + b_in + h @ w_hh^T + b_hh + sigma * n_t
    h'  = (1-a)*h + a*relu(pre)

For w_hh = d*I the recurrence is elementwise:
    h' = max(coef_a*h + v, coef_c*h)   with v = a*(x w^T + b + sigma n)

Sharding: 2 time-shards x 4 batch-shards. The step map is a 0.9-
contraction, so shard 1 starts 64 steps early from h=0 and the initial-
state error decays below 1e-3 by its owned range. Shard 0 runs from the
exact h0 (no warmup) and owns 544 steps; shard 1 owns 480 after the
64-step warmup, so both run the same 544-step program. This cuts the
serial DVE chain from 1024 to 544 steps/core at op width 128:
    544 * (60.4ns access charge + 128 * 1.042ns) = 105.5us  (the pacer)

PE: fp8e4m3 DoubleRow matmuls (2 k-tiles/pass, 0.5 cycles/row).
Pair-split quantization keeps bf16-level accuracy with 3 fp8 matmuls:
    u ~= x_hi@V0 + x_lo16@(V0/16) + x_hi@(W256 - V0)
(V0 = fp8(256*alpha*w_in); scales are powers of 2, folded into the
evacuation scale 32/256 and the DVE op's imm2 = 1/32.)

The noise (+biases, 32x prescale, fp8) is INJECTED INTO PSUM BY THE PE:
a DoubleRow identity matmul (kt0 = 8*I, kt1 = 0) opens each PSUM
accumulation group before the 6 x-matmuls. The noise tensor is a plain
prefetched load, so the only producer between the matmuls and the DVE
chain is the ACT evacuation (~1.1us) - no DMA latency (dge 650 + xfer
+ sem 900) sits on the critical path.

Per 16-step block (2 step-halves, each its own 2-bank PSUM tile and
its own v tile so the DVE sem-waits stay per-half):
  PE 2x28 DoubleRow matmuls 2.99us | ACT 2 evacs 2.07us
  Pool n-load + lagged cast-store 2.2us | SP x_hi+x_lo 1.46us
  DVE 16 fused step ops 3.10us  <- critical, runs stall-free

Rings: SP = x loads (+ wi/w0/w1 at startup, final bf16 half-stores),
Pool/SWDGE = noise prefetch + out cast-store (3-block lag so its sem
wait never blocks the sequencer) + w2 chunk, ACT = PSUM evacuation +
h0. Startup loads are spread across rings in first-use order because
a DMA blocks its ring sequencer while waiting and the ACT ring loses
~1.3us to the activation-table load.
"""

import os
import sys

import numpy as np

for _p in ("/opt/trn_rl_repo", os.path.expanduser("~/.axon_site/_ro/trn_rl_repo")):
    if os.path.isdir(_p) and _p not in sys.path:
        sys.path.insert(0, _p)

S, B, I, H = 1024, 128, 512, 512
TAU, DT = 100.0, 20.0
ALPHA = DT / TAU  # 0.2
SIGMA_REC = 0.05
SIGMA = float(np.sqrt(2.0 / ALPHA) * SIGMA_REC)

NCORES = 8
TSH = 2  # time shards
BSH = 4  # batch shards
BL = B // BSH  # 32 batch rows per core
WARM = 64  # warmup steps (0.9^64 ~ 1.2e-3 contraction of h-init error)
# Asymmetric split: shard 0 starts from the exact h0 (no warmup) and owns
# 544 steps; shard 1 owns 480 after a 64-step warmup, so both run the same
# 544-step program with no dead steps.
OWN0 = 544
STEPS = 544  # computed steps per core
SPB = 16  # steps per block
NB = STEPS // SPB  # 34 blocks
KC = I // 128  # 4 contraction chunks
HC = H // 128  # 4 hidden chunks
FB = SPB * BL  # 512 matmul free elems per block (sl, b)
VW = HC * BL  # 256: step op width (c, b)
NMM = 3  # fp8 matmul variants (hi@V0, lo@V1, hi@V2)
SW = 256.0  # weight prescale (power of 2)
NS = 32.0  # noise / v_sb prescale (power of 2)
EVAC_SCALE = NS / SW  # applied at PSUM evacuation
IMM2 = 1.0 / NS  # applied to v inside the DVE step op

_PROGRAM_CACHE: dict = {}
_CTRNN_OP = None


def _get_ctrnn_dve_op():
    """Fused DVE op: out = max(in0*s0 + in1*imm2, in0*s1)."""
    global _CTRNN_OP
    if _CTRNN_OP is not None:
        return _CTRNN_OP
    import concourse.dve_ops as dve_ops
    from concourse.dve_spec import Spec, Src0, Src1, _has_src1, lower, maxx
    from concourse.dve_spec import C0, C1, C2
    from concourse.dve_uop import DveOpSpec

    name = "CTRNN_STEP3_ANT"
    spec = Spec(
        body=maxx(Src0 * C0 + Src1 * C2, Src0 * C1),
        reference=lambda in0, in1, s0, s1, imm2: np.maximum(
            in0.astype(np.float32) * s0 + in1.astype(np.float32) * imm2,
            in0.astype(np.float32) * s1,
        ).astype(np.float32),
    )
    row = max(dve_ops._SUB_OPCODE_FOR_NAME.values()) + 1
    assert row < 0x20
    dve_ops._SUB_OPCODE_FOR_NAME[name] = row
    shas = {}
    for ver in ("v3", "v4"):
        try:
            shas[ver] = DveOpSpec(
                name=name, opcode=row, uops=lower(spec, ver=ver),
                rd1_en=_has_src1(spec),
            ).sha(ver)
        except Exception:
            pass
    op = dve_ops.DveOp(name, spec, subdim=False, uops_sha=shas)
    dve_ops.OPS.append(op)
    dve_ops.CUSTOM_DVE_SPECS[name] = spec
    _CTRNN_OP = op
    return op


def _build_program(n_blocks: int, coef_a: float, coef_c: float):
    import concourse.bacc as bacc
    import concourse.mybir as mybir
    from concourse import tile

    f32 = mybir.dt.float32
    bf16 = mybir.dt.bfloat16
    fp8 = mybir.dt.float8e4
    copy_fn = mybir.ActivationFunctionType.Copy
    add = mybir.AluOpType.add
    double_row = mybir.MatmulPerfMode.DoubleRow

    nc = bacc.Bacc(
        "TRN2",
        target_bir_lowering=False,
        debug=False,
        num_devices=NCORES,
    )

    xh_d = nc.dram_tensor("xh_t", [KC, 128, n_blocks, FB], fp8, kind="ExternalInput")
    xl_d = nc.dram_tensor("xl_t", [KC, 128, n_blocks, FB], fp8, kind="ExternalInput")
    # w_t[p, m, kcp, hc, kt, col] = Vm[hc*128+col, kcp*256 + kt*128 + p]
    w_d = nc.dram_tensor("w_t", [128, NMM, 2, HC, 2, 128], fp8, kind="ExternalInput")
    # wi_t: DoubleRow identity for the noise inject: kt0 = 8*I, kt1 = 0
    wi_d = nc.dram_tensor("wi_t", [128, 2, 128], fp8, kind="ExternalInput")
    # noise at 32x scale, half-major (hf, c, sl, b) layout + 256 zero pad
    # (the inject's kt1 rhs region is read x0 and must stay finite)
    n_d = nc.dram_tensor(
        "n_t", [n_blocks, 128, SPB * VW + 256], fp8, kind="ExternalInput"
    )
    h0_d = nc.dram_tensor("h0l", [128, VW], f32, kind="ExternalInput")
    o_d = nc.dram_tensor("out_l", [n_blocks, 128, SPB * VW], bf16, kind="ExternalOutput")

    WFREE = NMM * 2 * HC * 2 * 128  # 6144
    NPAD = SPB * VW + 256

    HSL = SPB // 2  # 8 steps per half
    HV = HSL * VW  # 1024 v elems per half

    with tile.TileContext(nc) as tc:
        with (
            tc.tile_pool(name="const", bufs=1) as cpool,
            tc.tile_pool(name="xp", bufs=4) as xpool,
            tc.tile_pool(name="pp", bufs=4, space="PSUM") as ppool,
            tc.tile_pool(name="vp", bufs=5) as vpool,
            tc.tile_pool(name="np", bufs=4) as npool,
            tc.tile_pool(name="op", bufs=5) as opool,
        ):
            # PE p-state pre-warm: ~3.5us of dummy DoubleRow matmuls on a
            # zeroed tile while the first loads are in flight, so the real
            # block-0 matmuls run at full clock instead of the ramp p-state.
            dm_sb = cpool.tile([128, 1024], fp8)
            nc.vector.memset(dm_sb[:], 0.0)
            ps_d = ppool.tile([128, HC * (FB // 2)], f32, name="psh")
            for _ in range(14):
                nc.tensor.matmul(
                    out=ps_d[:, :512],
                    lhsT=dm_sb[:, :256].rearrange("p (kt c) -> p kt c", kt=2),
                    rhs=dm_sb[:].rearrange("p (kt f) -> p kt f", kt=2),
                    start=True,
                    stop=True,
                    perf_mode=double_row,
                )

            # Startup loads spread across rings in first-use order (the ACT
            # sequencer is blocked ~1.3us by the activation-table load, so
            # only the last-needed pieces ride ACT). wi + m0 on SP ahead of
            # the x loads; m1 follows xh0 on SP; m2 + h0 on ACT.
            wi_sb = cpool.tile([128, 2 * 128], fp8)
            nc.sync.dma_start(
                out=wi_sb[:].rearrange("p (kt c) -> p kt c", kt=2), in_=wi_d.ap()
            )
            w_sb = cpool.tile([128, WFREE], fp8)
            wm = WFREE // NMM
            w_view_d = w_d.ap().rearrange("p m kcp hc kt c -> p (m kcp hc kt c)")
            nc.sync.dma_start(out=w_sb[:, 0:wm], in_=w_view_d[:, 0:wm])
            nc.sync.dma_start(out=w_sb[:, wm : 2 * wm], in_=w_view_d[:, wm : 2 * wm])
            h0_sb = cpool.tile([128, VW], f32)
            nc.scalar.dma_start(out=h0_sb[:], in_=h0_d.ap())

            w_view = w_sb[:].rearrange(
                "p (m kcp hc kt c) -> p m kcp hc kt c", m=NMM, kcp=2, hc=HC, kt=2
            )
            wi_view = wi_sb[:].rearrange("p (kt c) -> p kt c", kt=2)

            prev = h0_sb[:]
            pend_o: list = []  # (blk, tile) awaiting store, 2-block lag
            for blk in range(n_blocks):
                # ---- x block loads (SP ring); block 0 loads in step-half
                # pieces so the first matmul group is gated by 1KB transfers
                xh_sb = xpool.tile([128, KC * FB], fp8)
                xl_sb = xpool.tile([128, KC * FB], fp8)
                HFB0 = FB // 2
                if blk == 0:
                    for hx in range(2):
                        for xs, xd in ((xh_sb, xh_d), (xl_sb, xl_d)):
                            nc.sync.dma_start(
                                out=xs[:].rearrange("p (kc f) -> p kc f", kc=KC)[
                                    :, :, hx * HFB0 : (hx + 1) * HFB0
                                ],
                                in_=xd.ap()[:, :, blk, hx * HFB0 : (hx + 1) * HFB0]
                                .rearrange("kc p f -> p kc f"),
                            )
                else:
                    for xs, xd in ((xh_sb, xh_d), (xl_sb, xl_d)):
                        nc.sync.dma_start(
                            out=xs[:].rearrange("p (kc f) -> p kc f", kc=KC),
                            in_=xd.ap()[:, :, blk, :].rearrange("kc p f -> p kc f"),
                        )
                xh_view = xh_sb[:].rearrange("p (kc f) -> p kc f", kc=KC)
                xl_view = xl_sb[:].rearrange("p (kc f) -> p kc f", kc=KC)

                # ---- noise prefetch (Pool ring, plain load, off the
                # critical path; injected into PSUM by the PE)
                n_sb = npool.tile([128, NPAD], fp8)
                if blk == 0:
                    nc.gpsimd.dma_start(
                        out=n_sb[:, : HV + 256], in_=n_d.ap()[blk, :, : HV + 256]
                    )
                    nc.gpsimd.dma_start(
                        out=n_sb[:, HV + 256 :], in_=n_d.ap()[blk, :, HV + 256 :]
                    )
                else:
                    nc.gpsimd.dma_start(out=n_sb[:], in_=n_d.ap()[blk])
                if blk == 0:
                    nc.gpsimd.dma_start(
                        out=w_sb[:, 2 * wm : 3 * wm], in_=w_view_d[:, 2 * wm : 3 * wm]
                    )

                # ---- store a finished block (Pool/SWDGE fp32->bf16 cast)
                # with a 2-block lag: a DMA holds its sequencer through its
                # waits, so only issue it once its DVE block is complete
                if len(pend_o) >= 3:
                    sblk, stile = pend_o.pop(0)
                    nc.gpsimd.dma_start(out=o_d.ap()[sblk], in_=stile[:])

                # ---- per step-half: noise inject + 6 DoubleRow matmuls per
                # h-chunk, then evacuation. The PE opens each PSUM group by
                # writing 8*noise (DoubleRow identity), so the only producer
                # between matmuls and the DVE chain is the ACT evacuation
                # (~1.1us), and no DMA sits on the critical path.
                HFB = FB // 2  # 256 matmul free elems per half
                vh = []
                for h in range(2):
                    fsl = slice(h * HFB, (h + 1) * HFB)
                    ps = ppool.tile([128, HC * HFB], f32, name="psh")
                    for hc in range(HC):
                        noff = h * HV + hc * HFB
                        nc.tensor.matmul(
                            out=ps[:, hc * HFB : (hc + 1) * HFB],
                            lhsT=wi_view,
                            rhs=n_sb[:, noff : noff + 2 * HFB].rearrange(
                                "p (kt f) -> p kt f", kt=2
                            ),
                            start=True,
                            stop=False,
                            perf_mode=double_row,
                        )
                        idx = 0
                        for m, xv in ((0, xh_view), (1, xl_view), (2, xh_view)):
                            for kcp in range(2):
                                nc.tensor.matmul(
                                    out=ps[:, hc * HFB : (hc + 1) * HFB],
                                    lhsT=w_view[:, m, kcp, hc],
                                    rhs=xv[:, 2 * kcp : 2 * kcp + 2, fsl],
                                    start=False,
                                    stop=(idx == 5),
                                    perf_mode=double_row,
                                )
                                idx += 1
                    # evacuation (ACT): (c, sl, b) -> (sl, c, b), scale
                    # 32/256. One v tile per half keeps the DVE sem-waits
                    # per-half (a shared tile coarsens them to whole-block).
                    v_sb = vpool.tile([128, HV], bf16, name="v_h")
                    nc.scalar.activation(
                        out=v_sb[:].rearrange(
                            "p (sl c b) -> p c sl b", c=HC, b=BL
                        ),
                        in_=ps[:].rearrange("p (c sl b) -> p c sl b", c=HC, b=BL),
                        func=copy_fn,
                        scale=EVAC_SCALE,
                    )
                    vh.append(v_sb)

                # ---- recurrence: one fused DVE op per step (fp32 state;
                # the final block runs bf16 so its store needs no cast and
                # can drain on the idle SP ring)
                last = blk == n_blocks - 1
                o_sb = opool.tile([128, SPB * VW], bf16 if last else f32)
                for st in range(SPB):
                    osl = o_sb[:, st * VW : (st + 1) * VW]
                    vt = vh[st // HSL]
                    nc.vector._custom_dve(
                        _get_ctrnn_dve_op(), out=osl, in0=prev,
                        in1=vt[:, (st % HSL) * VW : (st % HSL + 1) * VW],
                        s0=coef_a, s1=coef_c, imm2=IMM2,
                    )
                    prev = osl
                pend_o.append((blk, o_sb))

            for sblk, stile in pend_o:
                if sblk == n_blocks - 1:
                    # halves: the first store drains during the last 8 DVE ops
                    hw_half = SPB * VW // 2
                    nc.sync.dma_start(
                        out=o_d.ap()[sblk, :, :hw_half], in_=stile[:, :hw_half]
                    )
                    nc.sync.dma_start(
                        out=o_d.ap()[sblk, :, hw_half:], in_=stile[:, hw_half:]
                    )
                else:
                    nc.gpsimd.dma_start(out=o_d.ap()[sblk], in_=stile[:])

    nc.finalize()
    return nc


def _get_program(n_blocks, coef_a, coef_c):
    key = (n_blocks, coef_a, coef_c)
    if key not in _PROGRAM_CACHE:
        _PROGRAM_CACHE[key] = _build_program(n_blocks, coef_a, coef_c)
    return _PROGRAM_CACHE[key]


def _f8():
    import ml_dtypes

    return np.dtype(ml_dtypes.float8_e4m3)


def _pack_weights(w_in):
    """Three fp8 matrices for the pair-split matmul, packed for DoubleRow.

    Returns w_t[p, m, kcp, hc, kt, col] = Vm[hc*128+col, kcp*256+kt*128+p].
    """
    f8 = _f8()
    w256 = (SW * ALPHA) * w_in.astype(np.float32)  # [H, I]
    v0 = w256.astype(f8)
    v0f = v0.astype(np.float32)
    v1 = (v0f / 16.0).astype(f8)
    v2 = (w256 - v0f).astype(f8)
    pack = np.stack([v0, v1, v2])  # [3, H, I]
    # [3, HC, col(128), kcp(2), kt(2), p(128)] -> [p, m, kcp, hc, kt, col]
    w_t = pack.reshape(NMM, HC, 128, 2, 2, 128).transpose(5, 0, 3, 1, 4, 2)
    return np.ascontiguousarray(w_t)


def _core_shards():
    return [(c // BSH, c % BSH) for c in range(NCORES)]  # (tau, beta)


def _pack_identity():
    """DoubleRow identity for the noise inject: kt0 = 8*I, kt1 = 0."""
    f8 = _f8()
    wi = np.zeros((128, 2, 128), dtype=np.float32)
    wi[np.arange(128), 0, np.arange(128)] = 8.0
    return wi.astype(f8)


def _host_inputs(x, noise, w_in, b_in, b_hh, h0):
    """Per-core input dicts (all layout + quantization work on the host)."""
    f8 = _f8()
    w_t = _pack_weights(w_in)
    wi_t = _pack_identity()

    xh_full = x.astype(f8)
    xl_full = ((x - xh_full.astype(np.float32)) * 16.0).astype(f8)
    bias = (NS * ALPHA) * (b_in + b_hh).astype(np.float32)
    nh_full = ((NS * ALPHA * SIGMA) * noise.astype(np.float32) + bias).astype(f8)

    in_maps = []
    for tau, beta in _core_shards():
        s0 = 0 if tau == 0 else OWN0 - WARM
        bs = slice(beta * BL, (beta + 1) * BL)

        def core_steps(xf):
            # steps [s0, s0+STEPS), zero-padded past the sequence end
            end = min(s0 + STEPS, S)
            seg = xf[s0:end, bs, :]
            if end - s0 < STEPS:
                pad = np.zeros(
                    (STEPS - (end - s0),) + seg.shape[1:], dtype=seg.dtype
                )
                seg = np.concatenate([seg, pad], axis=0)
            return seg

        def pack_x(xf):
            # x_c[kc, p, blk, (sl, b)] = xf[s0+blk*SPB+sl, b, kc*128+p]
            xc = (
                core_steps(xf)
                .reshape(NB, SPB, BL, I)
                .transpose(3, 0, 1, 2)
                .reshape(KC, 128, NB, FB)
            )
            return np.ascontiguousarray(xc)

        # n_c[blk, p, (hf, c, sl8, b)] = nh[s0+blk*SPB+hf*8+sl, b, c*128+p]
        # (half-major so the inject regions are contiguous and block 0 can
        # load its first half early), zero-padded
        n_c = (
            core_steps(nh_full)
            .reshape(NB, 2, SPB // 2, BL, HC, 128)
            .transpose(0, 5, 1, 4, 2, 3)
            .reshape(NB, 128, SPB * VW)
        )
        n_pad = np.zeros((NB, 128, SPB * VW + 256), dtype=f8)
        n_pad[:, :, : SPB * VW] = n_c
        if tau == 0:
            h0_l = (
                h0[bs].astype(np.float32).reshape(BL, HC, 128).transpose(2, 1, 0)
            )
            h0_l = np.ascontiguousarray(h0_l.reshape(128, VW))
        else:
            h0_l = np.zeros((128, VW), dtype=np.float32)
        in_maps.append(
            {
                "xh_t": pack_x(xh_full),
                "xl_t": pack_x(xl_full),
                "w_t": w_t,
                "wi_t": wi_t,
                "n_t": n_pad,
                "h0l": h0_l,
            }
        )
    return in_maps


def _gather_output(results):
    out = np.empty((S, B, H), dtype=np.float32)
    for c, (tau, beta) in enumerate(_core_shards()):
        o = np.asarray(results[c]["out_l"], dtype=np.float32)
        # o[blk, p, (sl, c, b)] -> steps[s, b, h=(hc, p)]
        o = (
            o.reshape(NB, 128, SPB, HC, BL)
            .transpose(0, 2, 4, 3, 1)
            .reshape(STEPS, BL, H)
        )
        if tau == 0:
            out[:OWN0, beta * BL : (beta + 1) * BL, :] = o[:OWN0]
        else:
            out[OWN0:, beta * BL : (beta + 1) * BL, :] = o[WARM : WARM + (S - OWN0)]
    return out


def _numpy_fallback(x, noise, w_in, b_in, w_hh, b_hh, h0):
    h = h0.astype(np.float32).copy()
    out = np.empty((S, B, H), dtype=np.float32)
    one_minus_a = np.float32(1.0 - ALPHA)
    a = np.float32(ALPHA)
    sg = np.float32(SIGMA)
    for t in range(S):
        pre = x[t] @ w_in.T + b_in + h @ w_hh.T + b_hh + sg * noise[t]
        h = h * one_minus_a + np.maximum(pre, 0) * a
        out[t] = h
    return out


def kernel(x, noise, w_in, b_in, w_hh, b_hh, h0):
    x = np.asarray(x, dtype=np.float32)
    noise = np.asarray(noise, dtype=np.float32)
    w_in = np.asarray(w_in, dtype=np.float32)
    b_in = np.asarray(b_in, dtype=np.float32)
    w_hh = np.asarray(w_hh, dtype=np.float32)
    b_hh = np.asarray(b_hh, dtype=np.float32)
    h0 = np.asarray(h0, dtype=np.float32)

    d = np.diagonal(w_hh)
    uniform_diag = np.all(w_hh == np.diag(d)) and np.all(d == d[0])
    if not uniform_diag:
        return _numpy_fallback(x, noise, w_in, b_in, w_hh, b_hh, h0)

    dval = float(d[0])
    coef_a = (1.0 - ALPHA) + ALPHA * dval  # 0.9 for d=0.5
    coef_c = 1.0 - ALPHA  # 0.8

    from concourse.bass_utils import run_bass_kernel_spmd

    nc = _get_program(NB, coef_a, coef_c)
    in_maps = _host_inputs(x, noise, w_in, b_in, b_hh, h0)
    res = run_bass_kernel_spmd(nc, in_maps, list(range(NCORES)))
    return _gather_output(res.results)
